# revision 1
# baseline (speedup 1.0000x reference)
"""BiMamba encoder Trainium2 kernel.

Data-parallel over batch (B=8) across 8 NeuronCores; each core runs the full
4-block (2 fwd + 2 bwd) BiMamba stack on one (T=1024, D=256) sequence in a
feature-major layout ([d on partitions, t on free dim]).

SSM scan uses the DVE tensor_tensor_scan HW op (fp32 internal state) per
(d-chunk, state-n): h[:,t] = dA[:,t]*h[:,t-1] + dBu[:,t].
"""

import sys

sys.path.insert(0, "/opt/trn_rl_repo")

import numpy as np

import concourse.bass as bass
import concourse.tile as tile
from concourse import mybir
from concourse.bass_utils import run_bass_kernel_spmd

# ---------------------------------------------------------------------------
# Monkeypatch: this walrus build's CTRL codegen accepts only ONE sync wait per
# instruction, but the Tile tail drain aggregates one wait per live semaphore.
# Split the waits across multiple drain instructions.
# ---------------------------------------------------------------------------
from concourse.tile import ScopedClock


def _patched_drain_and_barrier(self, tick_clock, wait_clock):
    nc = self.nc
    drain_inst = nc.sync.drain()
    wait_clock.add_sem_waits(
        drain_inst.ins, ScopedClock({None: tick_clock.global_clock})
    )
    si = drain_inst.ins.sync_info
    waits = list(si.on_wait or []) if si is not None else []
    MAXW = 1
    if len(waits) > MAXW:
        si.on_wait = waits[:MAXW]
        for i in range(MAXW, len(waits), MAXW):
            d2 = nc.sync.drain()
            si2 = d2.ins.sync_info
            if si2 is None:
                import bass_rust

                d2.ins.sync_info = bass_rust.SyncInfo(
                    on_wait=waits[i : i + MAXW], on_update=[]
                )
            else:
                si2.on_wait = waits[i : i + MAXW]
    nc.all_engine_barrier()
    assert self.sems is not None
    popped = nc._tile_sem_poison_stack.pop()
    assert popped is self._sem_poison
    nc.clear_and_free_semaphores(list(self.sems.allocated().values()))
    nc.all_engine_barrier()


tile.TileContext._drain_and_barrier = _patched_drain_and_barrier


def _split_multi_waits(nc, maxw=1):
    """This walrus build's codegen accepts at most one sync wait per
    instruction. Hoist extra waits onto preceding same-engine NoOps."""
    import bass_rust

    ctr = 0
    fn = nc.m.functions[0]
    for bb in fn.blocks:
        insts = list(bb.instructions)
        out = []
        changed = False
        for inst in insts:
            si = inst.sync_info
            waits = list(si.on_wait or []) if si is not None else []
            if len(waits) > maxw and inst.engine != mybir.EngineType.Unassigned:
                changed = True
                for i in range(0, len(waits) - maxw, maxw):
                    ctr += 1
                    nop = mybir.InstNoOp(name=f"wsplit-{ctr}", ins=[], outs=[])
                    nop.engine = inst.engine
                    nop.sync_info = bass_rust.SyncInfo(
                        on_wait=waits[i : i + maxw], on_update=[]
                    )
                    out.append(nop)
                si.on_wait = waits[len(waits) - maxw :]
            out.append(inst)
        if changed:
            bb.instructions = out


# ---------------------------------------------------------------------------

F32 = mybir.dt.float32
F16 = mybir.dt.float16
ALU = mybir.AluOpType
AF = mybir.ActivationFunctionType

D_MODEL = 256
D_STATE = 16
D_CONV = 4
DI = 512
DT_RANK = 16
NL = 2
NB = 4
T = 1024
NCORES = 8

NDC = DI // 128  # 4  d-chunks of the inner dim
NMC = D_MODEL // 128  # 2  d-chunks of the model dim
NT = T // 512  # 2  free-dim tiles for matmuls


def _build_program(ablate=(), split_waits=True, repeat=1):
    ablate = set(ablate)
    nc = bass.Bass("TRN2", target_bir_lowering=False, debug=False)

    if "identact" in ablate:
        _orig_activation = type(nc.scalar).activation

        def _ident_activation(self, out, in_, func, **kw):
            if func in (AF.Silu, AF.Exp, AF.Ln, AF.Square):
                func = AF.Identity
            return _orig_activation(self, out, in_, func, **kw)

        import types

        nc.scalar.activation = types.MethodType(_ident_activation, nc.scalar)

    # DRAM I/O
    xD = nc.dram_tensor("x", [T, D_MODEL], F32, kind="ExternalInput")
    iwTD = nc.dram_tensor("iwT", [NB, D_MODEL, 2 * DI], F16, kind="ExternalInput")
    cwD = nc.dram_tensor("cw", [NB, DI, D_CONV], F32, kind="ExternalInput")
    cbD = nc.dram_tensor("cb", [NB, DI, 1], F32, kind="ExternalInput")
    xwTD = nc.dram_tensor("xwT", [NB, DI, DT_RANK + 2 * D_STATE], F16, kind="ExternalInput")
    dtwTD = nc.dram_tensor("dtwT", [NB, DT_RANK, DI], F16, kind="ExternalInput")
    dtbD = nc.dram_tensor("dtb", [NB, DI, 1], F32, kind="ExternalInput")
    AD = nc.dram_tensor("A", [NB, DI, D_STATE], F32, kind="ExternalInput")
    DpD = nc.dram_tensor("Dp", [NB, DI, 1], F32, kind="ExternalInput")
    owTD = nc.dram_tensor("owT", [NB, DI, D_MODEL], F16, kind="ExternalInput")
    lngD = nc.dram_tensor("lng", [NB, D_MODEL, 1], F32, kind="ExternalInput")
    lnbD = nc.dram_tensor("lnb", [NB, D_MODEL, 1], F32, kind="ExternalInput")
    fwTD = nc.dram_tensor("fwT", [2 * D_MODEL, D_MODEL], F16, kind="ExternalInput")
    fbD = nc.dram_tensor("fb", [D_MODEL, 1], F32, kind="ExternalInput")
    identD = nc.dram_tensor("ident", [128, 128], F32, kind="ExternalInput")
    outD = nc.dram_tensor("out", [T, D_MODEL], F32, kind="ExternalOutput")

    with tile.TileContext(nc) as tc:
        from contextlib import ExitStack

        with ExitStack() as ctx:
            cpool = ctx.enter_context(tc.tile_pool(name="cpool", bufs=1))
            wpool = ctx.enter_context(tc.tile_pool(name="wpool", bufs=2))
            big = ctx.enter_context(tc.tile_pool(name="big", bufs=1))
            upool = ctx.enter_context(tc.tile_pool(name="upool", bufs=3))
            sc32 = ctx.enter_context(tc.tile_pool(name="sc32", bufs=2))
            w16 = ctx.enter_context(tc.tile_pool(name="w16", bufs=2))
            dpool = ctx.enter_context(tc.tile_pool(name="dpool", bufs=2, space="DRAM"))
            psA = ctx.enter_context(tc.tile_pool(name="psA", bufs=2, space="PSUM"))
            psB = ctx.enter_context(tc.tile_pool(name="psB", bufs=2, space="PSUM"))
            psS = ctx.enter_context(tc.tile_pool(name="psS", bufs=2, space="PSUM"))

            # --- constants ---
            ident_sb = cpool.tile([128, 128], F32, name="ident_sb")
            nc.sync.dma_start(ident_sb[:], identD.ap())
            ones_col = cpool.tile([128, 1], F32, name="ones_col")
            nc.gpsimd.memset(ones_col[:], 1.0)
            ones_row = cpool.tile([1, 128], F32, name="ones_row")
            nc.gpsimd.memset(ones_row[:], 1.0)
            eps_sb = cpool.tile([1, 1], F32, name="eps_sb")
            nc.gpsimd.memset(eps_sb[:], 1e-5)

            # --- load x in feature-major (reversed copy loaded later) ---
            xT_ap = xD.ap().transpose([1, 0])  # (256, 1024) strided view

            def load_x_fwd():
                x_fwd = []
                for c in range(NMC):
                    tfd = cpool.tile([128, T], F32, name=f"x_fwd{c}", tag="xio", bufs=2)
                    nc.sync.dma_start(tfd[:], xT_ap[c * 128 : (c + 1) * 128, :])
                    x_fwd.append(tfd)
                return x_fwd

            def emit_layer(u2, j):
                """One mamba block + residual + layernorm. u2: 2 chunk tiles
                [128, T] f32 (model dim). Returns new u2."""
                # ---- load weights for block j ----
                def wload(dram_ap, shape, tag, dtype=F32):
                    w = wpool.tile(shape, dtype, tag=tag, name=f"{tag}_{j}")
                    nc.sync.dma_start(w[:], dram_ap)
                    return w

                iwT_sb = [
                    wload(iwTD.ap()[j, kc * 128 : (kc + 1) * 128, :], [128, 2 * DI], f"iwT{kc}", F16)
                    for kc in range(NMC)
                ]
                cw_sb = [
                    wload(cwD.ap()[j, dc * 128 : (dc + 1) * 128, :], [128, D_CONV], f"cw{dc}")
                    for dc in range(NDC)
                ]
                cb_sb = [
                    wload(cbD.ap()[j, dc * 128 : (dc + 1) * 128, :], [128, 1], f"cb{dc}")
                    for dc in range(NDC)
                ]
                dtb_sb = [
                    wload(dtbD.ap()[j, dc * 128 : (dc + 1) * 128, :], [128, 1], f"dtb{dc}")
                    for dc in range(NDC)
                ]
                A_sb = [
                    wload(AD.ap()[j, dc * 128 : (dc + 1) * 128, :], [128, D_STATE], f"A{dc}")
                    for dc in range(NDC)
                ]
                D_sb = [
                    wload(DpD.ap()[j, dc * 128 : (dc + 1) * 128, :], [128, 1], f"Dp{dc}")
                    for dc in range(NDC)
                ]
                xwT_sb = [
                    wload(xwTD.ap()[j, dc * 128 : (dc + 1) * 128, :], [128, DT_RANK + 2 * D_STATE], f"xwT{dc}", F16)
                    for dc in range(NDC)
                ]
                owT_sb = [
                    wload(owTD.ap()[j, dc * 128 : (dc + 1) * 128, :], [128, D_MODEL], f"owT{dc}", F16)
                    for dc in range(NDC)
                ]
                dtwT_sb = wload(dtwTD.ap()[j], [DT_RANK, DI], "dtwT", F16)
                lng_sb = [
                    wload(lngD.ap()[j, mc * 128 : (mc + 1) * 128, :], [128, 1], f"lng{mc}")
                    for mc in range(NMC)
                ]
                lnb_sb = [
                    wload(lnbD.ap()[j, mc * 128 : (mc + 1) * 128, :], [128, 1], f"lnb{mc}")
                    for mc in range(NMC)
                ]

                # ---- in_proj: xz[m,:] = iw @ u, fused with conv+silu ----
                sz_sb = [big.tile([128, T], F16, tag=f"sz{dc}", name=f"sz{j}_{dc}") for dc in range(NDC)]
                xc_sb = [big.tile([128, T], F16, tag=f"xc{dc}", name=f"xc{j}_{dc}") for dc in range(NDC)]
                noprep = "noprep" in ablate
                if noprep:
                    dt16_sb = [big.tile([128, T], F16, tag=f"dt{dc}", name=f"dt{j}_{dc}") for dc in range(NDC)]
                    du16_sb = [big.tile([128, T], F16, tag=f"du{dc}", name=f"du{j}_{dc}") for dc in range(NDC)]
                    for dc in range(NDC):
                        nc.gpsimd.memset(sz_sb[dc][:], 0.01)
                        nc.gpsimd.memset(xc_sb[dc][:], 0.01)
                        nc.gpsimd.memset(dt16_sb[dc][:], 0.01)
                        nc.gpsimd.memset(du16_sb[dc][:], 0.01)
                    bcd = dpool.tile([2 * D_STATE, T], F16, tag="bcd", name=f"bcd{j}")
                # f16 copy of the residual stream for the f16 matmuls
                u16 = [big.tile([128, T], F16, tag=f"u16{mc}", name=f"u16{j}_{mc}") for mc in range(NMC)]
                if not noprep:
                    for mc in range(NMC):
                        nc.scalar.copy(u16[mc][:], u2[mc][:])
                for m in range(0 if noprep else 2 * DI // 128):  # 8 output chunks
                    px = psA.tile([128, T], F32, tag="psA", name=f"pxz{j}_{m}")
                    for nt in range(NT):
                        for kc in range(NMC):
                            nc.tensor.matmul(
                                px[:, nt * 512 : (nt + 1) * 512],
                                iwT_sb[kc][:, m * 128 : (m + 1) * 128],
                                u16[kc][:, nt * 512 : (nt + 1) * 512],
                                start=(kc == 0),
                                stop=(kc == NMC - 1),
                            )
                    if m < NDC:
                        # causal depthwise conv straight off PSUM, then silu
                        dc = m
                        acc = sc32.tile([128, T], F32, tag="dA", name=f"cacc{j}_{dc}")
                        nc.vector.tensor_scalar_mul(acc[:], px[:], cw_sb[dc][:, 3:4])
                        for k, sh in ((2, 1), (1, 2), (0, 3)):
                            nc.vector.scalar_tensor_tensor(
                                acc[:, sh:],
                                px[:, : T - sh],
                                cw_sb[dc][:, k : k + 1],
                                acc[:, sh:],
                                ALU.mult,
                                ALU.add,
                            )
                        nc.scalar.activation(
                            xc_sb[dc][:], acc[:], AF.Silu, bias=cb_sb[dc][:, 0:1]
                        )
                    else:
                        nc.scalar.activation(sz_sb[m - NDC][:], px[:], AF.Silu)

                # ---- x_proj: xdbl = xw @ xc  -> [48, T] (f16) ----
                NXP = DT_RANK + 2 * D_STATE  # 48
                if not noprep:
                    bc16_sb = big.tile([NXP, T], F16, tag="bc16", name=f"bc16{j}")
                    pxd = psA.tile([NXP, T], F32, tag="psA", name=f"pxd{j}")
                    for nt in range(NT):
                        for kc in range(NDC):
                            nc.tensor.matmul(
                                pxd[:, nt * 512 : (nt + 1) * 512],
                                xwT_sb[kc][:, :],
                                xc_sb[kc][:, nt * 512 : (nt + 1) * 512],
                                start=(kc == 0),
                                stop=(kc == NDC - 1),
                            )
                    nc.scalar.copy(bc16_sb[:], pxd[:])
                    # stage B/C rows in DRAM so the scan loop can DMA them with a
                    # partition-broadcast read (zero partition step needs a DRAM src)
                    bcd = dpool.tile([2 * D_STATE, T], F16, tag="bcd", name=f"bcd{j}")
                    nc.sync.dma_start(bcd[:], bc16_sb[DT_RANK : DT_RANK + 2 * D_STATE, :])

                    # ---- dt = softplus(dtw @ dt_in + dtb) ----
                    dt16_sb = [big.tile([128, T], F16, tag=f"dt{dc}", name=f"dt{j}_{dc}") for dc in range(NDC)]
                    du16_sb = [big.tile([128, T], F16, tag=f"du{dc}", name=f"du{j}_{dc}") for dc in range(NDC)]
                for dc in range(0 if noprep else NDC):
                    pdt = psA.tile([128, T], F32, tag="psA", name=f"pdt{j}_{dc}")
                    for nt in range(NT):
                        nc.tensor.matmul(
                            pdt[:, nt * 512 : (nt + 1) * 512],
                            dtwT_sb[:, dc * 128 : (dc + 1) * 128],
                            bc16_sb[0:DT_RANK, nt * 512 : (nt + 1) * 512],
                            start=True,
                            stop=True,
                        )
                    # softplus(x+b) = ln(exp(x+b) + 1) — no Softplus LUT in this
                    # compiler's ACT tables; exp+ln live in one table set.
                    nc.scalar.activation(
                        dt16_sb[dc][:], pdt[:], AF.Exp, bias=dtb_sb[dc][:, 0:1]
                    )
                    nc.scalar.activation(dt16_sb[dc][:], dt16_sb[dc][:], AF.Ln, bias=1.0)
                    nc.vector.tensor_mul(du16_sb[dc][:], dt16_sb[dc][:], xc_sb[dc][:])

                # ---- SSM scan over states ----
                y16_sb = [big.tile([128, T], F16, tag=f"yv{dc}", name=f"y16{j}_{dc}") for dc in range(NDC)]
                if "noscan" in ablate:
                    for dc in range(NDC):
                        nc.gpsimd.memset(y16_sb[dc][:], 0.01)
                nodma_reps = []
                if "nodma" in ablate:
                    for i in range(2):
                        br = w16.tile([128, T], F16, tag="brep", name=f"brepz{j}_{i}")
                        nc.gpsimd.memset(br[:], 0.01)
                        cr = w16.tile([128, T], F16, tag="crep", name=f"crepz{j}_{i}")
                        nc.gpsimd.memset(cr[:], 0.01)
                        nodma_reps.append((br, cr))
                for n in range(D_STATE if "noscan" not in ablate else 0):
                    if "nodma" in ablate:
                        brep, crep = nodma_reps[n % 2]
                    else:
                        brep = w16.tile([128, T], F16, tag="brep", name=f"brep{j}_{n}")
                        nc.sync.dma_start(
                            brep[:], bcd[n : n + 1, :].partition_broadcast(128)
                        )
                        crep = w16.tile([128, T], F16, tag="crep", name=f"crep{j}_{n}")
                        nc.sync.dma_start(
                            crep[:],
                            bcd[D_STATE + n : D_STATE + n + 1, :].partition_broadcast(128),
                        )
                    for dc in range(NDC):
                        if "noact" in ablate:
                            dA = dt16_sb[dc]
                        else:
                            dA = w16.tile([128, T], F16, tag="dA16", name=f"dA{j}_{n}_{dc}", bufs=3)
                            nc.scalar.activation(
                                dA[:], dt16_sb[dc][:], AF.Exp, scale=A_sb[dc][:, n : n + 1]
                            )
                        dBu = w16.tile([128, T], F16, tag="dBu", name=f"dBu{j}_{n}_{dc}")
                        nc.vector.tensor_mul(dBu[:], du16_sb[dc][:], brep[:])
                        h = w16.tile([128, T], F16, tag="h", name=f"h{j}_{n}_{dc}")
                        nc.vector.tensor_tensor_scan(
                            h[:], dA[:], dBu[:], 0.0, ALU.mult, ALU.add
                        )
                        if n == 0:
                            nc.vector.tensor_mul(y16_sb[dc][:], h[:], crep[:])
                        else:
                            m16 = w16.tile([128, T], F16, tag="m16", name=f"m{j}_{n}_{dc}")
                            nc.vector.tensor_mul(m16[:], h[:], crep[:])
                            nc.vector.tensor_add(y16_sb[dc][:], y16_sb[dc][:], m16[:])

                if "nopost" in ablate:
                    unew = [upool.tile([128, T], F32, tag=f"u{mc}", name=f"u{j}_{mc}") for mc in range(NMC)]
                    for mc in range(NMC):
                        nc.scalar.copy(unew[mc][:], u2[mc][:])
                    return unew

                # ---- D-skip + gate (all f16) ----
                yg_sb = [big.tile([128, T], F16, tag=f"yg{dc}", name=f"yg{j}_{dc}") for dc in range(NDC)]
                for dc in range(NDC):
                    nc.vector.scalar_tensor_tensor(
                        yg_sb[dc][:],
                        xc_sb[dc][:],
                        D_sb[dc][:, 0:1],
                        y16_sb[dc][:],
                        ALU.mult,
                        ALU.add,
                    )
                    nc.vector.tensor_mul(yg_sb[dc][:], yg_sb[dc][:], sz_sb[dc][:])

                # ---- out_proj + residual ----
                yr_sb = [big.tile([128, T], F32, tag=f"yv{mc}", name=f"yr{j}_{mc}") for mc in range(NMC)]
                for mc in range(NMC):
                    po = psA.tile([128, T], F32, tag="psA", name=f"po{j}_{mc}")
                    for nt in range(NT):
                        for kc in range(NDC):
                            nc.tensor.matmul(
                                po[:, nt * 512 : (nt + 1) * 512],
                                owT_sb[kc][:, mc * 128 : (mc + 1) * 128],
                                yg_sb[kc][:, nt * 512 : (nt + 1) * 512],
                                start=(kc == 0),
                                stop=(kc == NDC - 1),
                            )
                    nc.vector.tensor_add(yr_sb[mc][:], po[:], u2[mc][:])

                # ---- layernorm over the model dim (partition axis) ----
                sq_sb = [sc32.tile([128, T], F32, tag="dA", name=f"sq{j}_{mc}") for mc in range(NMC)]
                for mc in range(NMC):
                    nc.scalar.activation(sq_sb[mc][:], yr_sb[mc][:], AF.Square)
                mu_sb = cpool.tile([1, T], F32, name=f"mu{j}", tag="mu")
                ms_sb = cpool.tile([1, T], F32, name=f"ms{j}", tag="lnsm", bufs=3)
                for nt in range(NT):
                    s1 = psS.tile([1, 512], F32, tag="psS", name=f"s1{j}_{nt}")
                    for mc in range(NMC):
                        nc.tensor.matmul(
                            s1[:],
                            ones_col[:, 0:1],
                            yr_sb[mc][:, nt * 512 : (nt + 1) * 512],
                            start=(mc == 0),
                            stop=(mc == NMC - 1),
                        )
                    nc.scalar.mul(mu_sb[:, nt * 512 : (nt + 1) * 512], s1[:], 1.0 / D_MODEL)
                    s2 = psS.tile([1, 512], F32, tag="psS", name=f"s2{j}_{nt}")
                    for mc in range(NMC):
                        nc.tensor.matmul(
                            s2[:],
                            ones_col[:, 0:1],
                            sq_sb[mc][:, nt * 512 : (nt + 1) * 512],
                            start=(mc == 0),
                            stop=(mc == NMC - 1),
                        )
                    nc.scalar.mul(ms_sb[:, nt * 512 : (nt + 1) * 512], s2[:], 1.0 / D_MODEL)
                mu2_sb = cpool.tile([1, T], F32, name=f"mu2{j}", tag="lnsm", bufs=3)
                nc.scalar.activation(mu2_sb[:], mu_sb[:], AF.Square)
                var_sb = cpool.tile([1, T], F32, name=f"var{j}", tag="lnsm", bufs=3)
                nc.vector.tensor_tensor(var_sb[:], ms_sb[:], mu2_sb[:], ALU.subtract)
                # 1/sqrt(var+eps) = exp(-0.5*ln(var+eps)) — keeps ACT in the
                # ln/exp table set (no Sqrt table switch, no DVE reciprocal)
                sd_sb = cpool.tile([1, T], F32, name=f"sd{j}", tag="lnsm", bufs=3)
                nc.scalar.activation(sd_sb[:], var_sb[:], AF.Ln, bias=eps_sb[0:1, 0:1])
                inv_sb = cpool.tile([1, T], F32, name=f"inv{j}", tag="lnsm", bufs=3)
                nc.scalar.activation(inv_sb[:], sd_sb[:], AF.Exp, scale=-0.5)
                nmi_sb = cpool.tile([1, T], F32, name=f"nmi{j}", tag="lnsm", bufs=3)
                nc.vector.scalar_tensor_tensor(
                    nmi_sb[:], mu_sb[:], -1.0, inv_sb[:], ALU.mult, ALU.mult
                )
                unew = [upool.tile([128, T], F32, tag=f"u{mc}", name=f"u{j}_{mc}") for mc in range(NMC)]
                for nt in range(NT):
                    pinv = psB.tile([128, 512], F32, tag="psB", name=f"pinv{j}_{nt}")
                    nc.tensor.matmul(
                        pinv[:],
                        ones_row[0:1, :],
                        inv_sb[0:1, nt * 512 : (nt + 1) * 512],
                        start=True,
                        stop=True,
                    )
                    pnmi = psB.tile([128, 512], F32, tag="psB", name=f"pnmi{j}_{nt}")
                    nc.tensor.matmul(
                        pnmi[:],
                        ones_row[0:1, :],
                        nmi_sb[0:1, nt * 512 : (nt + 1) * 512],
                        start=True,
                        stop=True,
                    )
                    for mc in range(NMC):
                        sl = slice(nt * 512, (nt + 1) * 512)
                        nc.vector.tensor_mul(unew[mc][:, sl], yr_sb[mc][:, sl], pinv[:])
                        nc.vector.tensor_add(unew[mc][:, sl], unew[mc][:, sl], pnmi[:])
                for mc in range(NMC):
                    nc.scalar.activation(
                        unew[mc][:],
                        unew[mc][:],
                        AF.Identity,
                        bias=lnb_sb[mc][:, 0:1],
                        scale=lng_sb[mc][:, 0:1],
                    )
                return unew

            def emit_stack(u2, joff):
                for i in range(NL):
                    u2 = emit_layer(u2, joff + i)
                return u2

            for _rep in range(repeat):
                x_fwd = load_x_fwd()
                x_f = emit_stack(x_fwd, 0)

                # reversed input for the backward stack (reuses x_fwd slots)
                x_rev = []
                for c in range(NMC):
                    trv = cpool.tile([128, T], F32, name=f"x_rev{c}", tag="xio", bufs=2)
                    nc.sync.dma_start(trv[:], xT_ap[c * 128 : (c + 1) * 128, ::-1])
                    x_rev.append(trv)
                x_b_rev = emit_stack(x_rev, NL)
                # f16 copies for the fuse matmul (x_b also reversed back in time)
                x_b = []
                for c in range(NMC):
                    tb = cpool.tile([128, T], F16, name=f"x_b{c}", tag="xio16", bufs=4)
                    nc.vector.tensor_copy(tb[:], x_b_rev[c][:, ::-1])
                    x_b.append(tb)
                xf16 = []
                for c in range(NMC):
                    tf = cpool.tile([128, T], F16, name=f"xf16_{c}", tag="xio16", bufs=4)
                    nc.scalar.copy(tf[:], x_f[c][:])
                    xf16.append(tf)
                x_f = xf16

                # ---- fuse: out = fuse_w @ [x_f; x_b] + fb ----
                fwT_sb = []
                for kc in range(2 * NMC):
                    w = wpool.tile([128, D_MODEL], F16, name=f"fwT{kc}", tag=f"owT{kc}")
                    nc.sync.dma_start(w[:], fwTD.ap()[kc * 128 : (kc + 1) * 128, :])
                    fwT_sb.append(w)
                fb_sb = []
                for mc in range(NMC):
                    w = cpool.tile([128, 1], F32, name=f"fb{mc}")
                    nc.sync.dma_start(w[:], fbD.ap()[mc * 128 : (mc + 1) * 128, :])
                    fb_sb.append(w)
                xcat = x_f + x_b
                out_sb = []
                for mc in range(NMC):
                    pf = psA.tile([128, T], F32, tag="psA", name=f"pf{mc}")
                    for nt in range(NT):
                        for kc in range(2 * NMC):
                            nc.tensor.matmul(
                                pf[:, nt * 512 : (nt + 1) * 512],
                                fwT_sb[kc][:, mc * 128 : (mc + 1) * 128],
                                xcat[kc][:, nt * 512 : (nt + 1) * 512],
                                start=(kc == 0),
                                stop=(kc == 2 * NMC - 1),
                            )
                    o = sc32.tile([128, T], F32, name=f"out_sb{mc}", tag="dA")
                    nc.scalar.activation(o[:], pf[:], AF.Identity, bias=fb_sb[mc][:, 0:1])
                    out_sb.append(o)

                # ---- transpose to [T, D] and store ----
                for tt in range(T // 128):
                    ot = cpool.tile([128, D_MODEL], F32, name=f"outT{tt}", tag="outT")
                    for mc in range(NMC):
                        pt = psB.tile([128, 128], F32, tag="psB", name=f"pt{tt}_{mc}")
                        nc.tensor.transpose(
                            pt[:], out_sb[mc][:, tt * 128 : (tt + 1) * 128], ident_sb[:]
                        )
                        nc.scalar.copy(ot[:, mc * 128 : (mc + 1) * 128], pt[:])
                    nc.sync.dma_start(outD.ap()[tt * 128 : (tt + 1) * 128, :], ot[:])


    if split_waits:
        _split_multi_waits(nc)
    return nc


_NC_CACHE = None


def _get_program():
    global _NC_CACHE
    if _NC_CACHE is None:
        _NC_CACHE = _build_program()
    return _NC_CACHE


def _prep_weights(inputs):
    f = np.float32
    iw = np.asarray(inputs["in_proj_w"], f)  # (4, 1024, 256)
    cw = np.asarray(inputs["conv_w"], f)  # (4, 512, 1, 4)
    cb = np.asarray(inputs["conv_b"], f)  # (4, 512)
    xw = np.asarray(inputs["xproj_w"], f)  # (4, 48, 512)
    dtw = np.asarray(inputs["dtproj_w"], f)  # (4, 512, 16)
    dtb = np.asarray(inputs["dtproj_b"], f)  # (4, 512)
    alog = np.asarray(inputs["A_log"], f)  # (4, 512, 16)
    Dp = np.asarray(inputs["D"], f)  # (4, 512)
    ow = np.asarray(inputs["out_w"], f)  # (4, 256, 512)
    lng = np.asarray(inputs["ln_g"], f)  # (4, 256)
    lnb = np.asarray(inputs["ln_b"], f)  # (4, 256)
    fw = np.asarray(inputs["fuse_w"], f)  # (256, 512)
    fb = np.asarray(inputs["fuse_b"], f)  # (256,)
    c = np.ascontiguousarray
    h = np.float16
    return {
        "iwT": c(iw.transpose(0, 2, 1)).astype(h),  # (4, 256, 1024)
        "cw": c(cw[:, :, 0, :]),  # (4, 512, 4)
        "cb": c(cb[:, :, None]),
        "xwT": c(xw.transpose(0, 2, 1)).astype(h),  # (4, 512, 48)
        "dtwT": c(dtw.transpose(0, 2, 1)).astype(h),  # (4, 16, 512)
        "dtb": c(dtb[:, :, None]),
        "A": c(-np.exp(alog)),  # (4, 512, 16)
        "Dp": c(Dp[:, :, None]),
        "owT": c(ow.transpose(0, 2, 1)).astype(h),  # (4, 512, 256)
        "lng": c(lng[:, :, None]),
        "lnb": c(lnb[:, :, None]),
        "fwT": c(fw.T).astype(h),  # (512, 256)
        "fb": c(fb[:, None]),
        "ident": np.eye(128, dtype=f),
    }


LAST_RUN = None


def kernel(**inputs) -> np.ndarray:
    global LAST_RUN
    import os

    nc = _get_program()
    x = np.asarray(inputs["x"], np.float32)  # (8, 1024, 256)
    assert x.shape == (NCORES, T, D_MODEL)
    w = _prep_weights(inputs)
    in_maps = [
        {"x": np.ascontiguousarray(x[i]), **w} for i in range(NCORES)
    ]
    trace = bool(int(os.environ.get("BIMAMBA_TRACE", "0")))
    res = run_bass_kernel_spmd(
        nc, in_maps, core_ids=list(range(NCORES)), trace=trace
    )
    LAST_RUN = res
    out = np.stack([res.results[i]["out"] for i in range(NCORES)], axis=0)
    return out.astype(np.float32)


if __name__ == "__main__":
    # quick CoreSim numeric check against the jax reference
    import importlib.util
    import jax

    spec = importlib.util.spec_from_file_location("reference", "/root/problem/reference.py")
    ref = importlib.util.module_from_spec(spec)
    spec.loader.exec_module(ref)
    with jax.default_device(jax.devices("cpu")[0]):
        inputs = {k: np.asarray(v) for k, v in ref.setup_inputs().items()}
        expected = np.asarray(jax.jit(ref.reference, backend="cpu")(**inputs))

    from concourse.bass_interp import CoreSim, Direction, InstructionExecutor

    _orig_act = InstructionExecutor.visit_InstActivation

    def _patched_act(self, instruction, *args, **kwargs):
        f = instruction.func
        if f not in (AF.Silu, AF.Softplus):
            return _orig_act(self, instruction, *args, **kwargs)
        instruction.func = AF.Identity
        try:
            r = _orig_act(self, instruction, *args, **kwargs)
        finally:
            instruction.func = f
        out_ap = instruction.outs[0]
        view = self.view_ap(out_ap, Direction.WRITE, instruction)
        x = np.asarray(view[...], dtype=np.float64)
        if f == AF.Silu:
            y = x / (1.0 + np.exp(-x))
        else:
            y = np.logaddexp(0.0, x)
        view[:] = y
        return r

    InstructionExecutor.visit_InstActivation = _patched_act

    nc = _build_program(split_waits=False)
    w = _prep_weights(inputs)
    sim = CoreSim(nc)
    core = 0
    sim.tensor("x")[:] = np.ascontiguousarray(inputs["x"][core])
    for k, v in w.items():
        sim.tensor(k)[:] = v
    sim.simulate()
    got = sim.tensor("out")
    exp = expected[core]
    denom = np.abs(exp).max()
    err = np.abs(got - exp).max() / denom
    print("core0 absmax rel err:", err)



# revision 9
# speedup vs baseline: 81.6585x; 81.6585x over previous
"""BiMamba encoder Trainium2 kernel.

Data-parallel over batch (B=8) across 8 NeuronCores; each core runs the full
4-block (2 fwd + 2 bwd) BiMamba stack on one (T=1024, D=256) sequence in a
feature-major layout ([d on partitions, t on free dim]).

Engine plan (v2):
- SSM scans (the only sequential op) on DVE via tensor_tensor_scan (f16).
- dBu / h*C elementwise muls split DVE <-> Pool by a routing knob (Pool runs
  plain tensor_tensor at ~4x cost but is otherwise idle).
- y = sum_n h_n*C_n accumulated on the PE via identity matmuls into PSUM;
  the D-skip rides the same accumulation as a diag(D) matmul, and the
  residual add rides the out_proj matmul as an identity matmul.
- Causal depthwise conv = 4 shifted diag(w_k) matmuls on PE from a
  zero-padded f16 copy of the in_proj output.
- Residual stream kept in f16 end to end.

PSUM budget (8 banks): A0, C0 = [128,1024] f32 (2 banks each) for matmul
outputs; Y0, Y1 = [128,1024] f32 (2 banks each) as the scan-loop y
accumulators for a dc-pair at a time (the scan loop runs dc-pairs
sequentially), reused for conv psum / LN broadcasts / fuse transposes.
"""

import sys

sys.path.insert(0, "/opt/trn_rl_repo")

import numpy as np

import concourse.bass as bass
import concourse.tile as tile
from concourse import mybir
from concourse.bass_utils import run_bass_kernel_spmd

# ---------------------------------------------------------------------------
# Monkeypatch: this walrus build's CTRL codegen accepts only ONE sync wait per
# instruction, but the Tile tail drain aggregates one wait per live semaphore.
# Split the waits across multiple drain instructions.
# ---------------------------------------------------------------------------
from concourse.tile import ScopedClock


def _patched_drain_and_barrier(self, tick_clock, wait_clock):
    nc = self.nc
    drain_inst = nc.sync.drain()
    wait_clock.add_sem_waits(
        drain_inst.ins, ScopedClock({None: tick_clock.global_clock})
    )
    si = drain_inst.ins.sync_info
    waits = list(si.on_wait or []) if si is not None else []
    MAXW = 1
    if len(waits) > MAXW:
        si.on_wait = waits[:MAXW]
        for i in range(MAXW, len(waits), MAXW):
            d2 = nc.sync.drain()
            si2 = d2.ins.sync_info
            if si2 is None:
                import bass_rust

                d2.ins.sync_info = bass_rust.SyncInfo(
                    on_wait=waits[i : i + MAXW], on_update=[]
                )
            else:
                si2.on_wait = waits[i : i + MAXW]
    nc.all_engine_barrier()
    assert self.sems is not None
    popped = nc._tile_sem_poison_stack.pop()
    assert popped is self._sem_poison
    nc.clear_and_free_semaphores(list(self.sems.allocated().values()))
    nc.all_engine_barrier()


tile.TileContext._drain_and_barrier = _patched_drain_and_barrier


def _split_multi_waits(nc, maxw=1):
    """This walrus build's codegen accepts at most one sync wait per
    instruction. Hoist extra waits onto preceding same-engine NoOps."""
    import bass_rust

    ctr = 0
    fn = nc.m.functions[0]
    for bb in fn.blocks:
        insts = list(bb.instructions)
        out = []
        changed = False
        for inst in insts:
            si = inst.sync_info
            waits = list(si.on_wait or []) if si is not None else []
            if len(waits) > maxw and inst.engine != mybir.EngineType.Unassigned:
                changed = True
                for i in range(0, len(waits) - maxw, maxw):
                    ctr += 1
                    nop = mybir.InstNoOp(name=f"wsplit-{ctr}", ins=[], outs=[])
                    nop.engine = inst.engine
                    nop.sync_info = bass_rust.SyncInfo(
                        on_wait=waits[i : i + maxw], on_update=[]
                    )
                    out.append(nop)
                si.on_wait = waits[len(waits) - maxw :]
            out.append(inst)
        if changed:
            bb.instructions = out


# ---------------------------------------------------------------------------

F32 = mybir.dt.float32
F16 = mybir.dt.float16
ALU = mybir.AluOpType
AF = mybir.ActivationFunctionType

D_MODEL = 256
D_STATE = 16
D_CONV = 4
DI = 512
DT_RANK = 16
NL = 2
NB = 4
T = 1024
NCORES = 8

NDC = DI // 128  # 4  d-chunks of the inner dim
NMC = D_MODEL // 128  # 2  d-chunks of the model dim
NT = T // 512  # 2  free-dim tiles for matmuls

# Routing knobs: of every 16 ops in a group, how many go to Pool (gpsimd).
# Pool tensor_tensor runs at ~4x DVE cost but is otherwise idle while DVE
# carries the (serial-only-on-DVE) scans.
POOL16 = {"dBu": 13, "hC": 13, "du": 16}


def _build_program(ablate=(), split_waits=True, repeat=1):
    ablate = set(ablate)
    nc = bass.Bass("TRN2", target_bir_lowering=False, debug=False)

    # DRAM I/O
    x16D = nc.dram_tensor("x16", [T, D_MODEL], F16, kind="ExternalInput")
    iwTD = nc.dram_tensor("iwT", [NB, D_MODEL, 2 * DI], F16, kind="ExternalInput")
    cbD = nc.dram_tensor("cb", [NB, DI, 1], F32, kind="ExternalInput")
    dcwD = nc.dram_tensor("dcw", [NB, NDC, D_CONV, 128, 128], F16, kind="ExternalInput")
    ddD = nc.dram_tensor("dd", [NB, NDC, 128, 128], F16, kind="ExternalInput")
    xwTD = nc.dram_tensor("xwT", [NB, DI, DT_RANK + 2 * D_STATE], F16, kind="ExternalInput")
    dtwTD = nc.dram_tensor("dtwT", [NB, DT_RANK, DI], F16, kind="ExternalInput")
    dtbD = nc.dram_tensor("dtb", [NB, DI, 1], F32, kind="ExternalInput")
    AD = nc.dram_tensor("A", [NB, DI, D_STATE], F32, kind="ExternalInput")
    owTD = nc.dram_tensor("owT", [NB, DI, D_MODEL], F16, kind="ExternalInput")
    lngD = nc.dram_tensor("lng", [NB, D_MODEL, 1], F32, kind="ExternalInput")
    lnbD = nc.dram_tensor("lnb", [NB, D_MODEL, 1], F32, kind="ExternalInput")
    fwTD = nc.dram_tensor("fwT", [2 * D_MODEL, D_MODEL], F16, kind="ExternalInput")
    fbD = nc.dram_tensor("fb", [D_MODEL, 1], F32, kind="ExternalInput")
    identD = nc.dram_tensor("ident", [128, 128], F32, kind="ExternalInput")
    identhD = nc.dram_tensor("identh", [128, 128], F16, kind="ExternalInput")
    outD = nc.dram_tensor("out", [T, D_MODEL], F32, kind="ExternalOutput")

    # Pool routing (deterministic round-robin per group)
    _rr = {}

    def tt(out, a, b, op, group):
        k = POOL16.get(group, 0)
        i = _rr.get(group, 0)
        _rr[group] = i + 1
        eng = nc.gpsimd if (i % 16) < k else nc.vector
        eng.tensor_tensor(out, a, b, op)

    with tile.TileContext(nc) as tc:
        from contextlib import ExitStack

        with ExitStack() as ctx:
            cpool = ctx.enter_context(tc.tile_pool(name="cpool", bufs=1))
            wpool = ctx.enter_context(tc.tile_pool(name="wpool", bufs=2))
            big = ctx.enter_context(tc.tile_pool(name="big", bufs=1))
            upool = ctx.enter_context(tc.tile_pool(name="upool", bufs=3))
            sc32 = ctx.enter_context(tc.tile_pool(name="sc32", bufs=1))
            w16 = ctx.enter_context(tc.tile_pool(name="w16", bufs=2))
            dpool = ctx.enter_context(tc.tile_pool(name="dpool", bufs=2, space="DRAM"))
            # 4 PSUM slots x 2 banks = all 8 banks
            ps = ctx.enter_context(tc.tile_pool(name="ps", bufs=1, space="PSUM"))

            def pstile(tag, name):
                return ps.tile([128, T], F32, tag=tag, name=name)

            # --- constants ---
            ident_sb = cpool.tile([128, 128], F32, name="ident_sb")
            nc.sync.dma_start(ident_sb[:], identD.ap())
            identh_sb = cpool.tile([128, 128], F16, name="identh_sb")
            nc.sync.dma_start(identh_sb[:], identhD.ap())
            ones_col = cpool.tile([128, 1], F32, name="ones_col")
            nc.gpsimd.memset(ones_col[:], 1.0)
            ones_row = cpool.tile([1, 128], F32, name="ones_row")
            nc.gpsimd.memset(ones_row[:], 1.0)
            eps_sb = cpool.tile([1, 1], F32, name="eps_sb")
            nc.gpsimd.memset(eps_sb[:], 1e-5)

            x16T_ap = x16D.ap().transpose([1, 0])  # (256, 1024) strided view

            def load_x(reverse):
                xs = []
                for c in range(NMC):
                    tfd = cpool.tile(
                        [128, T], F16, name=f"x_{'r' if reverse else 'f'}{c}",
                        tag="xio", bufs=4,
                    )
                    src = x16T_ap[c * 128 : (c + 1) * 128, :]
                    nc.sync.dma_start(tfd[:], src[:, ::-1] if reverse else src)
                    xs.append(tfd)
                return xs

            def emit_layer(u16, j):
                """One mamba block + residual + layernorm.
                u16: NMC f16 [128,T] tiles (model dim). Returns new u16."""

                # ---- load weights for block j ----
                def wload(dram_ap, shape, tag, dtype=F32):
                    w = wpool.tile(shape, dtype, tag=tag, name=f"{tag}_{j}")
                    nc.sync.dma_start(w[:], dram_ap)
                    return w

                iwT_sb = [
                    wload(iwTD.ap()[j, kc * 128 : (kc + 1) * 128, :], [128, 2 * DI], f"iwT{kc}", F16)
                    for kc in range(NMC)
                ]
                cb_sb = [
                    wload(cbD.ap()[j, dc * 128 : (dc + 1) * 128, :], [128, 1], f"cb{dc}")
                    for dc in range(NDC)
                ]
                dcw_sb = [
                    [
                        wload(dcwD.ap()[j, dc, k], [128, 128], f"dcw{dc}_{k}", F16)
                        for k in range(D_CONV)
                    ]
                    for dc in range(NDC)
                ]
                dd_sb = [
                    wload(ddD.ap()[j, dc], [128, 128], f"dd{dc}", F16)
                    for dc in range(NDC)
                ]
                dtb_sb = [
                    wload(dtbD.ap()[j, dc * 128 : (dc + 1) * 128, :], [128, 1], f"dtb{dc}")
                    for dc in range(NDC)
                ]
                A_sb = [
                    wload(AD.ap()[j, dc * 128 : (dc + 1) * 128, :], [128, D_STATE], f"A{dc}")
                    for dc in range(NDC)
                ]
                xwT_sb = [
                    wload(xwTD.ap()[j, dc * 128 : (dc + 1) * 128, :], [128, DT_RANK + 2 * D_STATE], f"xwT{dc}", F16)
                    for dc in range(NDC)
                ]
                owT_sb = [
                    wload(owTD.ap()[j, dc * 128 : (dc + 1) * 128, :], [128, D_MODEL], f"owT{dc}", F16)
                    for dc in range(NDC)
                ]
                dtwT_sb = wload(dtwTD.ap()[j], [DT_RANK, DI], "dtwT", F16)
                lng_sb = [
                    wload(lngD.ap()[j, mc * 128 : (mc + 1) * 128, :], [128, 1], f"lng{mc}")
                    for mc in range(NMC)
                ]
                lnb_sb = [
                    wload(lnbD.ap()[j, mc * 128 : (mc + 1) * 128, :], [128, 1], f"lnb{mc}")
                    for mc in range(NMC)
                ]

                # ---- in_proj + conv(PE) + silu ----
                sz_sb = [big.tile([128, T], F16, tag=f"sz{dc}", name=f"sz{j}_{dc}") for dc in range(NDC)]
                xc_sb = [big.tile([128, T], F16, tag=f"xc{dc}", name=f"xc{j}_{dc}") for dc in range(NDC)]
                for m in range(2 * DI // 128):  # 8 output chunks
                    px = pstile("A0" if m % 2 == 0 else "C0", f"pxz{j}_{m}")
                    for nt in range(NT):
                        for kc in range(NMC):
                            nc.tensor.matmul(
                                px[:, nt * 512 : (nt + 1) * 512],
                                iwT_sb[kc][:, m * 128 : (m + 1) * 128],
                                u16[kc][:, nt * 512 : (nt + 1) * 512],
                                start=(kc == 0),
                                stop=(kc == NMC - 1),
                            )
                    if m < NDC:
                        dc = m
                        # zero-padded f16 copy of xin, then 4 shifted diag
                        # matmuls on PE implement the causal depthwise conv
                        xin = big.tile([128, T + D_CONV - 1], F16, tag="xin", bufs=2, name=f"xin{j}_{dc}")
                        nc.gpsimd.memset(xin[:, 0 : D_CONV - 1], 0.0)
                        nc.scalar.copy(xin[:, D_CONV - 1 :], px[:])
                        pc = pstile("Y0" if m % 2 == 0 else "Y1", f"pc{j}_{dc}")
                        for nt in range(NT):
                            for k in range(D_CONV):
                                nc.tensor.matmul(
                                    pc[:, nt * 512 : (nt + 1) * 512],
                                    dcw_sb[dc][k][:],
                                    xin[:, nt * 512 + k : nt * 512 + k + 512],
                                    start=(k == 0),
                                    stop=(k == D_CONV - 1),
                                )
                        nc.scalar.activation(
                            xc_sb[dc][:], pc[:], AF.Silu, bias=cb_sb[dc][:, 0:1]
                        )
                    else:
                        nc.scalar.activation(sz_sb[m - NDC][:], px[:], AF.Silu)

                # ---- x_proj: xdbl = xw @ xc  -> [48, T] (f16) ----
                NXP = DT_RANK + 2 * D_STATE  # 48
                bc16_sb = big.tile([NXP, T], F16, tag="bc16", name=f"bc16{j}")
                pxd_full = pstile("A0", f"pxd{j}")
                pxd = pxd_full[0:NXP, :]
                for nt in range(NT):
                    for kc in range(NDC):
                        nc.tensor.matmul(
                            pxd[:, nt * 512 : (nt + 1) * 512],
                            xwT_sb[kc][:, :],
                            xc_sb[kc][:, nt * 512 : (nt + 1) * 512],
                            start=(kc == 0),
                            stop=(kc == NDC - 1),
                        )
                    nc.scalar.copy(
                        bc16_sb[:, nt * 512 : (nt + 1) * 512],
                        pxd[:, nt * 512 : (nt + 1) * 512],
                    )
                # stage B/C rows in DRAM so the scan loop can DMA them with a
                # partition-broadcast read (zero partition step needs DRAM src)
                bcd = dpool.tile([2 * D_STATE, T], F16, tag="bcd", name=f"bcd{j}")
                nc.sync.dma_start(bcd[:], bc16_sb[DT_RANK : DT_RANK + 2 * D_STATE, :])

                # ---- dt = softplus(dtw @ dt_in + dtb); du = dt * xc ----
                dt16_sb = [big.tile([128, T], F16, tag=f"dt{dc}", name=f"dt{j}_{dc}") for dc in range(NDC)]
                du16_sb = [big.tile([128, T], F16, tag=f"du{dc}", name=f"du{j}_{dc}") for dc in range(NDC)]
                for dc in range(NDC):
                    pdt = pstile("A0" if dc % 2 == 0 else "C0", f"pdt{j}_{dc}")
                    for nt in range(NT):
                        nc.tensor.matmul(
                            pdt[:, nt * 512 : (nt + 1) * 512],
                            dtwT_sb[:, dc * 128 : (dc + 1) * 128],
                            bc16_sb[0:DT_RANK, nt * 512 : (nt + 1) * 512],
                            start=True,
                            stop=True,
                        )
                    # softplus(x+b) = ln(exp(x+b) + 1) — exp+ln live in one
                    # ACT table set (no Softplus LUT in this compiler)
                    nc.scalar.activation(
                        dt16_sb[dc][:], pdt[:], AF.Exp, bias=dtb_sb[dc][:, 0:1]
                    )
                    nc.scalar.activation(dt16_sb[dc][:], dt16_sb[dc][:], AF.Ln, bias=1.0)
                    tt(du16_sb[dc][:], dt16_sb[dc][:], xc_sb[dc][:], ALU.mult, "du")

                # ---- SSM scan: dc-pairs, y accumulated on PE into PSUM ----
                yg_sb = [big.tile([128, T], F16, tag=f"yg{dc}", name=f"yg{j}_{dc}") for dc in range(NDC)]
                for pair in range(NDC // 2):
                    dcs = (2 * pair, 2 * pair + 1)
                    py = {dc: pstile("Y0" if dc % 2 == 0 else "Y1", f"py{j}_{dc}") for dc in dcs}
                    for n in range(D_STATE):
                        brep = w16.tile([128, T], F16, tag="brep", name=f"brep{j}_{pair}_{n}", bufs=3)
                        nc.sync.dma_start(
                            brep[:], bcd[n : n + 1, :].partition_broadcast(128)
                        )
                        crep = w16.tile([128, T], F16, tag="crep", name=f"crep{j}_{pair}_{n}", bufs=3)
                        nc.sync.dma_start(
                            crep[:],
                            bcd[D_STATE + n : D_STATE + n + 1, :].partition_broadcast(128),
                        )
                        for dc in dcs:
                            dA = w16.tile([128, T], F16, tag=f"dA{dc % 2}", name=f"dA{j}_{n}_{dc}", bufs=3)
                            nc.scalar.activation(
                                dA[:], dt16_sb[dc][:], AF.Exp, scale=A_sb[dc][:, n : n + 1]
                            )
                            dBu = w16.tile([128, T], F16, tag="dBu", name=f"dBu{j}_{n}_{dc}", bufs=3)
                            tt(dBu[:], du16_sb[dc][:], brep[:], ALU.mult, "dBu")
                            h = w16.tile([128, T], F16, tag="h", name=f"h{j}_{n}_{dc}", bufs=3)
                            nc.vector.tensor_tensor_scan(
                                h[:], dA[:], dBu[:], 0.0, ALU.mult, ALU.add
                            )
                            hC = w16.tile([128, T], F16, tag="hC", name=f"hC{j}_{n}_{dc}", bufs=3)
                            tt(hC[:], h[:], crep[:], ALU.mult, "hC")
                            for nt in range(NT):
                                sl = slice(nt * 512, (nt + 1) * 512)
                                nc.tensor.matmul(
                                    py[dc][:, sl],
                                    identh_sb[:],
                                    hC[:, sl],
                                    start=(n == 0),
                                    stop=False,
                                )
                    for dc in dcs:
                        # D-skip rides the same accumulation: py += diag(D)*xc
                        for nt in range(NT):
                            sl = slice(nt * 512, (nt + 1) * 512)
                            nc.tensor.matmul(
                                py[dc][:, sl], dd_sb[dc][:], xc_sb[dc][:, sl],
                                start=False, stop=True,
                            )
                        # gate
                        nc.vector.tensor_tensor(
                            yg_sb[dc][:], py[dc][:], sz_sb[dc][:], ALU.mult
                        )

                # ---- out_proj + residual (residual rides PE as I @ u16) ----
                yr_sb = [sc32.tile([128, T], F32, tag=f"yr{mc}", name=f"yr{j}_{mc}") for mc in range(NMC)]
                sq_sb = [sc32.tile([128, T], F32, tag=f"sq{mc}", name=f"sq{j}_{mc}") for mc in range(NMC)]
                po_tiles = []
                for mc in range(NMC):
                    po = pstile("A0" if mc == 0 else "C0", f"po{j}_{mc}")
                    po_tiles.append(po)
                    for nt in range(NT):
                        sl = slice(nt * 512, (nt + 1) * 512)
                        for kc in range(NDC):
                            nc.tensor.matmul(
                                po[:, sl],
                                owT_sb[kc][:, mc * 128 : (mc + 1) * 128],
                                yg_sb[kc][:, sl],
                                start=(kc == 0),
                                stop=False,
                            )
                        nc.tensor.matmul(
                            po[:, sl], identh_sb[:], u16[mc][:, sl],
                            start=False, stop=True,
                        )
                    nc.scalar.copy(yr_sb[mc][:], po[:])
                    nc.scalar.activation(sq_sb[mc][:], yr_sb[mc][:], AF.Square)

                # ---- layernorm over the model dim (partition axis) ----
                s1 = pstile("Y0", f"s1{j}")
                s2 = pstile("Y1", f"s2{j}")
                mu_sb = cpool.tile([1, T], F32, name=f"mu{j}", tag="mu", bufs=1)
                ms_sb = cpool.tile([1, T], F32, name=f"ms{j}", tag="lnsm", bufs=3)
                for nt in range(NT):
                    sl = slice(nt * 512, (nt + 1) * 512)
                    for mc in range(NMC):
                        nc.tensor.matmul(
                            s1[0:1, sl], ones_col[:, 0:1], yr_sb[mc][:, sl],
                            start=(mc == 0), stop=(mc == NMC - 1),
                        )
                    for mc in range(NMC):
                        nc.tensor.matmul(
                            s2[0:1, sl], ones_col[:, 0:1], sq_sb[mc][:, sl],
                            start=(mc == 0), stop=(mc == NMC - 1),
                        )
                nc.scalar.mul(mu_sb[:], s1[0:1, :], 1.0 / D_MODEL)
                nc.scalar.mul(ms_sb[:], s2[0:1, :], 1.0 / D_MODEL)
                mu2_sb = cpool.tile([1, T], F32, name=f"mu2{j}", tag="lnsm", bufs=3)
                nc.scalar.activation(mu2_sb[:], mu_sb[:], AF.Square)
                var_sb = cpool.tile([1, T], F32, name=f"var{j}", tag="lnsm", bufs=3)
                nc.vector.tensor_tensor(var_sb[:], ms_sb[:], mu2_sb[:], ALU.subtract)
                # 1/sqrt(var+eps) = exp(-0.5*ln(var+eps)) — stays in the
                # ln/exp ACT table set
                sd_sb = cpool.tile([1, T], F32, name=f"sd{j}", tag="lnsm", bufs=3)
                nc.scalar.activation(sd_sb[:], var_sb[:], AF.Ln, bias=eps_sb[0:1, 0:1])
                inv_sb = cpool.tile([1, T], F32, name=f"inv{j}", tag="lnsm", bufs=3)
                nc.scalar.activation(inv_sb[:], sd_sb[:], AF.Exp, scale=-0.5)
                nmi_sb = cpool.tile([1, T], F32, name=f"nmi{j}", tag="lnsm", bufs=3)
                nc.vector.scalar_tensor_tensor(
                    nmi_sb[:], mu_sb[:], -1.0, inv_sb[:], ALU.mult, ALU.mult
                )
                pinv = pstile("Y0", f"pinv{j}")
                pnmi = pstile("Y1", f"pnmi{j}")
                for nt in range(NT):
                    sl = slice(nt * 512, (nt + 1) * 512)
                    nc.tensor.matmul(
                        pinv[:, sl], ones_row[0:1, :], inv_sb[0:1, sl],
                        start=True, stop=True,
                    )
                    nc.tensor.matmul(
                        pnmi[:, sl], ones_row[0:1, :], nmi_sb[0:1, sl],
                        start=True, stop=True,
                    )
                unew = [upool.tile([128, T], F16, tag=f"u{mc}", name=f"u{j}_{mc}") for mc in range(NMC)]
                utmp = [sc32.tile([128, T], F32, tag=f"ut{mc}", name=f"ut{j}_{mc}") for mc in range(NMC)]
                for mc in range(NMC):
                    nc.vector.tensor_tensor(utmp[mc][:], yr_sb[mc][:], pinv[:], ALU.mult)
                    nc.vector.tensor_tensor(utmp[mc][:], utmp[mc][:], pnmi[:], ALU.add)
                    nc.scalar.activation(
                        unew[mc][:],
                        utmp[mc][:],
                        AF.Identity,
                        bias=lnb_sb[mc][:, 0:1],
                        scale=lng_sb[mc][:, 0:1],
                    )
                return unew

            def emit_stack(u16s, joff):
                for i in range(NL):
                    u16s = emit_layer(u16s, joff + i)
                return u16s

            for _rep in range(repeat):
                x_f = emit_stack(load_x(False), 0)
                x_b_rev = emit_stack(load_x(True), NL)
                # reverse the bwd stack output back in time
                x_b = []
                for c in range(NMC):
                    tb = cpool.tile([128, T], F16, name=f"x_b{c}", tag="xio16", bufs=2)
                    nc.vector.tensor_copy(tb[:], x_b_rev[c][:, ::-1])
                    x_b.append(tb)

                # ---- fuse: out = fuse_w @ [x_f; x_b] + fb ----
                fwT_sb = []
                for kc in range(2 * NMC):
                    w = wpool.tile([128, D_MODEL], F16, name=f"fwT{kc}", tag=f"owT{kc}")
                    nc.sync.dma_start(w[:], fwTD.ap()[kc * 128 : (kc + 1) * 128, :])
                    fwT_sb.append(w)
                fb_sb = []
                for mc in range(NMC):
                    w = cpool.tile([128, 1], F32, name=f"fb{mc}", tag="fb", bufs=2)
                    nc.sync.dma_start(w[:], fbD.ap()[mc * 128 : (mc + 1) * 128, :])
                    fb_sb.append(w)
                xcat = x_f + x_b
                out_sb = []
                for mc in range(NMC):
                    pf = pstile("A0" if mc == 0 else "C0", f"pf{mc}")
                    for nt in range(NT):
                        for kc in range(2 * NMC):
                            nc.tensor.matmul(
                                pf[:, nt * 512 : (nt + 1) * 512],
                                fwT_sb[kc][:, mc * 128 : (mc + 1) * 128],
                                xcat[kc][:, nt * 512 : (nt + 1) * 512],
                                start=(kc == 0),
                                stop=(kc == 2 * NMC - 1),
                            )
                    o = sc32.tile([128, T], F32, name=f"out_sb{mc}", tag=f"yr{mc}")
                    nc.scalar.activation(o[:], pf[:], AF.Identity, bias=fb_sb[mc][:, 0:1])
                    out_sb.append(o)

                # ---- transpose to [T, D] and store ----
                pt_slots = [pstile("Y0", "ptY0"), pstile("Y1", "ptY1")]
                for tt_i in range(T // 128):
                    ot = cpool.tile([128, D_MODEL], F32, name=f"outT{tt_i}", tag="outT", bufs=4)
                    for mc in range(NMC):
                        pt = pt_slots[(tt_i * NMC + mc) % 2][:, 0:128]
                        nc.tensor.transpose(
                            pt, out_sb[mc][:, tt_i * 128 : (tt_i + 1) * 128], ident_sb[:]
                        )
                        nc.scalar.copy(ot[:, mc * 128 : (mc + 1) * 128], pt)
                    nc.sync.dma_start(outD.ap()[tt_i * 128 : (tt_i + 1) * 128, :], ot[:])

    if split_waits:
        _split_multi_waits(nc)
    return nc


_NC_CACHE = None


def _get_program():
    global _NC_CACHE
    if _NC_CACHE is None:
        _NC_CACHE = _build_program()
    return _NC_CACHE


def _prep_weights(inputs):
    f = np.float32
    iw = np.asarray(inputs["in_proj_w"], f)  # (4, 1024, 256)
    cw = np.asarray(inputs["conv_w"], f)  # (4, 512, 1, 4)
    cb = np.asarray(inputs["conv_b"], f)  # (4, 512)
    xw = np.asarray(inputs["xproj_w"], f)  # (4, 48, 512)
    dtw = np.asarray(inputs["dtproj_w"], f)  # (4, 512, 16)
    dtb = np.asarray(inputs["dtproj_b"], f)  # (4, 512)
    alog = np.asarray(inputs["A_log"], f)  # (4, 512, 16)
    Dp = np.asarray(inputs["D"], f)  # (4, 512)
    ow = np.asarray(inputs["out_w"], f)  # (4, 256, 512)
    lng = np.asarray(inputs["ln_g"], f)  # (4, 256)
    lnb = np.asarray(inputs["ln_b"], f)  # (4, 256)
    fw = np.asarray(inputs["fuse_w"], f)  # (256, 512)
    fb = np.asarray(inputs["fuse_b"], f)  # (256,)
    c = np.ascontiguousarray
    h = np.float16
    # conv taps and D as diag matrices for the PE (stationary operands)
    dcw = np.zeros((NB, NDC, D_CONV, 128, 128), h)
    dd = np.zeros((NB, NDC, 128, 128), h)
    idx = np.arange(128)
    for j in range(NB):
        for dc in range(NDC):
            dsl = slice(dc * 128, (dc + 1) * 128)
            for k in range(D_CONV):
                dcw[j, dc, k, idx, idx] = cw[j, dsl, 0, k].astype(h)
            dd[j, dc, idx, idx] = Dp[j, dsl].astype(h)
    return {
        "iwT": c(iw.transpose(0, 2, 1)).astype(h),  # (4, 256, 1024)
        "cb": c(cb[:, :, None]),
        "dcw": dcw,
        "dd": dd,
        "xwT": c(xw.transpose(0, 2, 1)).astype(h),  # (4, 512, 48)
        "dtwT": c(dtw.transpose(0, 2, 1)).astype(h),  # (4, 16, 512)
        "dtb": c(dtb[:, :, None]),
        "A": c(-np.exp(alog)),  # (4, 512, 16)
        "owT": c(ow.transpose(0, 2, 1)).astype(h),  # (4, 512, 256)
        "lng": c(lng[:, :, None]),
        "lnb": c(lnb[:, :, None]),
        "fwT": c(fw.T).astype(h),  # (512, 256)
        "fb": c(fb[:, None]),
        "ident": np.eye(128, dtype=f),
        "identh": np.eye(128, dtype=h),
    }


LAST_RUN = None


def kernel(**inputs) -> np.ndarray:
    global LAST_RUN
    import os

    nc = _get_program()
    x = np.asarray(inputs["x"], np.float32)  # (8, 1024, 256)
    assert x.shape == (NCORES, T, D_MODEL)
    w = _prep_weights(inputs)
    in_maps = [
        {"x16": np.ascontiguousarray(x[i]).astype(np.float16), **w}
        for i in range(NCORES)
    ]
    trace = bool(int(os.environ.get("BIMAMBA_TRACE", "0")))
    res = run_bass_kernel_spmd(
        nc, in_maps, core_ids=list(range(NCORES)), trace=trace
    )
    LAST_RUN = res
    out = np.stack([res.results[i]["out"] for i in range(NCORES)], axis=0)
    return out.astype(np.float32)


if __name__ == "__main__":
    # quick CoreSim numeric check against the jax reference
    import importlib.util
    import jax

    spec = importlib.util.spec_from_file_location("reference", "/root/problem/reference.py")
    ref = importlib.util.module_from_spec(spec)
    spec.loader.exec_module(ref)
    with jax.default_device(jax.devices("cpu")[0]):
        inputs = {k: np.asarray(v) for k, v in ref.setup_inputs().items()}
        expected = np.asarray(jax.jit(ref.reference, backend="cpu")(**inputs))

    from concourse.bass_interp import CoreSim, Direction, InstructionExecutor

    _orig_act = InstructionExecutor.visit_InstActivation

    def _patched_act(self, instruction, *args, **kwargs):
        f = instruction.func
        if f not in (AF.Silu, AF.Softplus):
            return _orig_act(self, instruction, *args, **kwargs)
        instruction.func = AF.Identity
        try:
            r = _orig_act(self, instruction, *args, **kwargs)
        finally:
            instruction.func = f
        out_ap = instruction.outs[0]
        view = self.view_ap(out_ap, Direction.WRITE, instruction)
        x = np.asarray(view[...], dtype=np.float64)
        if f == AF.Silu:
            y = x / (1.0 + np.exp(-x))
        else:
            y = np.logaddexp(0.0, x)
        view[:] = y
        return r

    InstructionExecutor.visit_InstActivation = _patched_act

    nc = _build_program(split_waits=False)
    w = _prep_weights(inputs)
    sim = CoreSim(nc)
    core = 0
    sim.tensor("x16")[:] = np.ascontiguousarray(inputs["x"][core]).astype(np.float16)
    for k, v in w.items():
        sim.tensor(k)[:] = v
    sim.simulate()
    got = sim.tensor("out")
    exp = expected[core]
    denom = np.abs(exp).max()
    err = np.abs(got - exp).max() / denom
    print("core0 absmax rel err:", err)


# revision 10
# speedup vs baseline: 135.3622x; 1.6577x over previous
"""BiMamba encoder Trainium2 kernel.

Data-parallel over batch (B=8) across 8 NeuronCores; each core runs the full
4-block (2 fwd + 2 bwd) BiMamba stack on one (T=1024, D=256) sequence in a
feature-major layout ([d on partitions, t on free dim]).

Engine plan (v2):
- SSM scans (the only sequential op) on DVE via tensor_tensor_scan (f16).
- dBu / h*C elementwise muls split DVE <-> Pool by a routing knob (Pool runs
  plain tensor_tensor at ~4x cost but is otherwise idle).
- y = sum_n h_n*C_n accumulated on the PE via identity matmuls into PSUM;
  the D-skip rides the same accumulation as a diag(D) matmul, and the
  residual add rides the out_proj matmul as an identity matmul.
- Causal depthwise conv = 4 shifted diag(w_k) matmuls on PE from a
  zero-padded f16 copy of the in_proj output.
- Residual stream kept in f16 end to end.

PSUM budget (8 banks): A0, C0 = [128,1024] f32 (2 banks each) for matmul
outputs; Y0, Y1 = [128,1024] f32 (2 banks each) as the scan-loop y
accumulators for a dc-pair at a time (the scan loop runs dc-pairs
sequentially), reused for conv psum / LN broadcasts / fuse transposes.
"""

import sys

sys.path.insert(0, "/opt/trn_rl_repo")

import numpy as np

import concourse.bass as bass
import concourse.tile as tile
from concourse import mybir
from concourse.bass_utils import run_bass_kernel_spmd

# ---------------------------------------------------------------------------
# Monkeypatch: this walrus build's CTRL codegen accepts only ONE sync wait per
# instruction, but the Tile tail drain aggregates one wait per live semaphore.
# Split the waits across multiple drain instructions.
# ---------------------------------------------------------------------------
from concourse.tile import ScopedClock


def _patched_drain_and_barrier(self, tick_clock, wait_clock):
    nc = self.nc
    drain_inst = nc.sync.drain()
    wait_clock.add_sem_waits(
        drain_inst.ins, ScopedClock({None: tick_clock.global_clock})
    )
    si = drain_inst.ins.sync_info
    waits = list(si.on_wait or []) if si is not None else []
    MAXW = 1
    if len(waits) > MAXW:
        si.on_wait = waits[:MAXW]
        for i in range(MAXW, len(waits), MAXW):
            d2 = nc.sync.drain()
            si2 = d2.ins.sync_info
            if si2 is None:
                import bass_rust

                d2.ins.sync_info = bass_rust.SyncInfo(
                    on_wait=waits[i : i + MAXW], on_update=[]
                )
            else:
                si2.on_wait = waits[i : i + MAXW]
    nc.all_engine_barrier()
    assert self.sems is not None
    popped = nc._tile_sem_poison_stack.pop()
    assert popped is self._sem_poison
    nc.clear_and_free_semaphores(list(self.sems.allocated().values()))
    nc.all_engine_barrier()


tile.TileContext._drain_and_barrier = _patched_drain_and_barrier


def _split_multi_waits(nc, maxw=1):
    """This walrus build's codegen accepts at most one sync wait per
    instruction. Hoist extra waits onto preceding same-engine NoOps."""
    import bass_rust

    ctr = 0
    fn = nc.m.functions[0]
    for bb in fn.blocks:
        insts = list(bb.instructions)
        out = []
        changed = False
        for inst in insts:
            si = inst.sync_info
            waits = list(si.on_wait or []) if si is not None else []
            if len(waits) > maxw and inst.engine != mybir.EngineType.Unassigned:
                changed = True
                for i in range(0, len(waits) - maxw, maxw):
                    ctr += 1
                    nop = mybir.InstNoOp(name=f"wsplit-{ctr}", ins=[], outs=[])
                    nop.engine = inst.engine
                    nop.sync_info = bass_rust.SyncInfo(
                        on_wait=waits[i : i + maxw], on_update=[]
                    )
                    out.append(nop)
                si.on_wait = waits[len(waits) - maxw :]
            out.append(inst)
        if changed:
            bb.instructions = out


# ---------------------------------------------------------------------------

F32 = mybir.dt.float32
F16 = mybir.dt.float16
ALU = mybir.AluOpType
AF = mybir.ActivationFunctionType

D_MODEL = 256
D_STATE = 16
D_CONV = 4
DI = 512
DT_RANK = 16
NL = 2
NB = 4
T = 1024
NCORES = 8

NDC = DI // 128  # 4  d-chunks of the inner dim
NMC = D_MODEL // 128  # 2  d-chunks of the model dim
NT = T // 512  # 2  free-dim tiles for matmuls

# Routing knobs: of every 16 ops in a group, how many go to Pool (gpsimd).
# Pool tensor_tensor runs at ~4x DVE cost but is otherwise idle while DVE
# carries the (serial-only-on-DVE) scans.
POOL16 = {"dBu": 0, "hC": 0, "du": 0}


def _build_program(ablate=(), split_waits=True, repeat=1):
    ablate = set(ablate)
    nc = bass.Bass("TRN2", target_bir_lowering=False, debug=False)

    # DRAM I/O
    x16D = nc.dram_tensor("x16", [T, D_MODEL], F16, kind="ExternalInput")
    iwTD = nc.dram_tensor("iwT", [NB, D_MODEL, 2 * DI], F16, kind="ExternalInput")
    cbD = nc.dram_tensor("cb", [NB, DI, 1], F32, kind="ExternalInput")
    dcwD = nc.dram_tensor("dcw", [NB, NDC, D_CONV, 128, 128], F16, kind="ExternalInput")
    ddD = nc.dram_tensor("dd", [NB, NDC, 128, 128], F16, kind="ExternalInput")
    xwTD = nc.dram_tensor("xwT", [NB, DI, DT_RANK + 2 * D_STATE], F16, kind="ExternalInput")
    dtwTD = nc.dram_tensor("dtwT", [NB, DT_RANK, DI], F16, kind="ExternalInput")
    dtbD = nc.dram_tensor("dtb", [NB, DI, 1], F32, kind="ExternalInput")
    AD = nc.dram_tensor("A", [NB, DI, D_STATE], F32, kind="ExternalInput")
    owTD = nc.dram_tensor("owT", [NB, DI, D_MODEL], F16, kind="ExternalInput")
    lngD = nc.dram_tensor("lng", [NB, D_MODEL, 1], F32, kind="ExternalInput")
    lnbD = nc.dram_tensor("lnb", [NB, D_MODEL, 1], F32, kind="ExternalInput")
    fwTD = nc.dram_tensor("fwT", [2 * D_MODEL, D_MODEL], F16, kind="ExternalInput")
    fbD = nc.dram_tensor("fb", [D_MODEL, 1], F32, kind="ExternalInput")
    identD = nc.dram_tensor("ident", [128, 128], F32, kind="ExternalInput")
    identhD = nc.dram_tensor("identh", [128, 128], F16, kind="ExternalInput")
    outD = nc.dram_tensor("out", [T, D_MODEL], F32, kind="ExternalOutput")

    # Pool routing (deterministic round-robin per group)
    _rr = {}

    def tt(out, a, b, op, group):
        k = POOL16.get(group, 0)
        i = _rr.get(group, 0)
        _rr[group] = i + 1
        eng = nc.gpsimd if (i % 16) < k else nc.vector
        eng.tensor_tensor(out, a, b, op)

    with tile.TileContext(nc) as tc:
        from contextlib import ExitStack

        with ExitStack() as ctx:
            cpool = ctx.enter_context(tc.tile_pool(name="cpool", bufs=1))
            wpool = ctx.enter_context(tc.tile_pool(name="wpool", bufs=2))
            big = ctx.enter_context(tc.tile_pool(name="big", bufs=1))
            upool = ctx.enter_context(tc.tile_pool(name="upool", bufs=3))
            sc32 = ctx.enter_context(tc.tile_pool(name="sc32", bufs=1))
            w16 = ctx.enter_context(tc.tile_pool(name="w16", bufs=2))
            dpool = ctx.enter_context(tc.tile_pool(name="dpool", bufs=2, space="DRAM"))
            # 4 PSUM slots x 2 banks = all 8 banks
            ps = ctx.enter_context(tc.tile_pool(name="ps", bufs=1, space="PSUM"))

            def pstile(tag, name):
                return ps.tile([128, T], F32, tag=tag, name=name)

            # --- constants ---
            ident_sb = cpool.tile([128, 128], F32, name="ident_sb")
            nc.sync.dma_start(ident_sb[:], identD.ap())
            identh_sb = cpool.tile([128, 128], F16, name="identh_sb")
            nc.sync.dma_start(identh_sb[:], identhD.ap())
            ones_col = cpool.tile([128, 1], F32, name="ones_col")
            nc.gpsimd.memset(ones_col[:], 1.0)
            ones_row = cpool.tile([1, 128], F32, name="ones_row")
            nc.gpsimd.memset(ones_row[:], 1.0)
            eps_sb = cpool.tile([1, 1], F32, name="eps_sb")
            nc.gpsimd.memset(eps_sb[:], 1e-5)

            x16T_ap = x16D.ap().transpose([1, 0])  # (256, 1024) strided view

            def load_x(reverse):
                xs = []
                for c in range(NMC):
                    tfd = cpool.tile(
                        [128, T], F16, name=f"x_{'r' if reverse else 'f'}{c}",
                        tag="xio", bufs=4,
                    )
                    src = x16T_ap[c * 128 : (c + 1) * 128, :]
                    nc.sync.dma_start(tfd[:], src[:, ::-1] if reverse else src)
                    xs.append(tfd)
                return xs

            def emit_layer(u16, j):
                """One mamba block + residual + layernorm.
                u16: NMC f16 [128,T] tiles (model dim). Returns new u16."""

                # ---- load weights for block j ----
                def wload(dram_ap, shape, tag, dtype=F32):
                    w = wpool.tile(shape, dtype, tag=tag, name=f"{tag}_{j}")
                    nc.sync.dma_start(w[:], dram_ap)
                    return w

                iwT_sb = [
                    wload(iwTD.ap()[j, kc * 128 : (kc + 1) * 128, :], [128, 2 * DI], f"iwT{kc}", F16)
                    for kc in range(NMC)
                ]
                cb_sb = [
                    wload(cbD.ap()[j, dc * 128 : (dc + 1) * 128, :], [128, 1], f"cb{dc}")
                    for dc in range(NDC)
                ]
                dcw_sb = [
                    [
                        wload(dcwD.ap()[j, dc, k], [128, 128], f"dcw{dc}_{k}", F16)
                        for k in range(D_CONV)
                    ]
                    for dc in range(NDC)
                ]
                dd_sb = [
                    wload(ddD.ap()[j, dc], [128, 128], f"dd{dc}", F16)
                    for dc in range(NDC)
                ]
                dtb_sb = [
                    wload(dtbD.ap()[j, dc * 128 : (dc + 1) * 128, :], [128, 1], f"dtb{dc}")
                    for dc in range(NDC)
                ]
                A_sb = [
                    wload(AD.ap()[j, dc * 128 : (dc + 1) * 128, :], [128, D_STATE], f"A{dc}")
                    for dc in range(NDC)
                ]
                xwT_sb = [
                    wload(xwTD.ap()[j, dc * 128 : (dc + 1) * 128, :], [128, DT_RANK + 2 * D_STATE], f"xwT{dc}", F16)
                    for dc in range(NDC)
                ]
                owT_sb = [
                    wload(owTD.ap()[j, dc * 128 : (dc + 1) * 128, :], [128, D_MODEL], f"owT{dc}", F16)
                    for dc in range(NDC)
                ]
                dtwT_sb = wload(dtwTD.ap()[j], [DT_RANK, DI], "dtwT", F16)
                lng_sb = [
                    wload(lngD.ap()[j, mc * 128 : (mc + 1) * 128, :], [128, 1], f"lng{mc}")
                    for mc in range(NMC)
                ]
                lnb_sb = [
                    wload(lnbD.ap()[j, mc * 128 : (mc + 1) * 128, :], [128, 1], f"lnb{mc}")
                    for mc in range(NMC)
                ]

                # ---- in_proj + conv(PE) + silu ----
                sz_sb = [big.tile([128, T], F16, tag=f"sz{dc}", name=f"sz{j}_{dc}") for dc in range(NDC)]
                xc_sb = [big.tile([128, T], F16, tag=f"xc{dc}", name=f"xc{j}_{dc}") for dc in range(NDC)]
                for m in range(2 * DI // 128):  # 8 output chunks
                    px = pstile("A0" if m % 2 == 0 else "C0", f"pxz{j}_{m}")
                    for nt in range(NT):
                        for kc in range(NMC):
                            nc.tensor.matmul(
                                px[:, nt * 512 : (nt + 1) * 512],
                                iwT_sb[kc][:, m * 128 : (m + 1) * 128],
                                u16[kc][:, nt * 512 : (nt + 1) * 512],
                                start=(kc == 0),
                                stop=(kc == NMC - 1),
                            )
                    if m < NDC:
                        dc = m
                        # zero-padded f16 copy of xin, then 4 shifted diag
                        # matmuls on PE implement the causal depthwise conv
                        xin = big.tile([128, T + D_CONV - 1], F16, tag="xin", bufs=2, name=f"xin{j}_{dc}")
                        nc.gpsimd.memset(xin[:, 0 : D_CONV - 1], 0.0)
                        nc.scalar.copy(xin[:, D_CONV - 1 :], px[:])
                        pc = pstile("Y0" if m % 2 == 0 else "Y1", f"pc{j}_{dc}")
                        for nt in range(NT):
                            for k in range(D_CONV):
                                nc.tensor.matmul(
                                    pc[:, nt * 512 : (nt + 1) * 512],
                                    dcw_sb[dc][k][:],
                                    xin[:, nt * 512 + k : nt * 512 + k + 512],
                                    start=(k == 0),
                                    stop=(k == D_CONV - 1),
                                )
                        nc.scalar.activation(
                            xc_sb[dc][:], pc[:], AF.Silu, bias=cb_sb[dc][:, 0:1]
                        )
                    else:
                        nc.scalar.activation(sz_sb[m - NDC][:], px[:], AF.Silu)

                # ---- x_proj: xdbl = xw @ xc  -> [48, T] (f16) ----
                NXP = DT_RANK + 2 * D_STATE  # 48
                bc16_sb = big.tile([NXP, T], F16, tag="bc16", name=f"bc16{j}")
                pxd_full = pstile("A0", f"pxd{j}")
                pxd = pxd_full[0:NXP, :]
                for nt in range(NT):
                    for kc in range(NDC):
                        nc.tensor.matmul(
                            pxd[:, nt * 512 : (nt + 1) * 512],
                            xwT_sb[kc][:, :],
                            xc_sb[kc][:, nt * 512 : (nt + 1) * 512],
                            start=(kc == 0),
                            stop=(kc == NDC - 1),
                        )
                    nc.scalar.copy(
                        bc16_sb[:, nt * 512 : (nt + 1) * 512],
                        pxd[:, nt * 512 : (nt + 1) * 512],
                    )
                # stage B/C rows in DRAM so the scan loop can DMA them with a
                # partition-broadcast read (zero partition step needs DRAM src)
                bcd = dpool.tile([2 * D_STATE, T], F16, tag="bcd", name=f"bcd{j}")
                nc.sync.dma_start(bcd[:], bc16_sb[DT_RANK : DT_RANK + 2 * D_STATE, :])

                # ---- dt = softplus(dtw @ dt_in + dtb); du = dt * xc ----
                dt16_sb = [big.tile([128, T], F16, tag=f"dt{dc}", name=f"dt{j}_{dc}") for dc in range(NDC)]
                du16_sb = [big.tile([128, T], F16, tag=f"du{dc}", name=f"du{j}_{dc}") for dc in range(NDC)]
                for dc in range(NDC):
                    pdt = pstile("A0" if dc % 2 == 0 else "C0", f"pdt{j}_{dc}")
                    for nt in range(NT):
                        nc.tensor.matmul(
                            pdt[:, nt * 512 : (nt + 1) * 512],
                            dtwT_sb[:, dc * 128 : (dc + 1) * 128],
                            bc16_sb[0:DT_RANK, nt * 512 : (nt + 1) * 512],
                            start=True,
                            stop=True,
                        )
                    # softplus(x+b) = ln(exp(x+b) + 1) — exp+ln live in one
                    # ACT table set (no Softplus LUT in this compiler)
                    nc.scalar.activation(
                        dt16_sb[dc][:], pdt[:], AF.Exp, bias=dtb_sb[dc][:, 0:1]
                    )
                    nc.scalar.activation(dt16_sb[dc][:], dt16_sb[dc][:], AF.Ln, bias=1.0)
                    tt(du16_sb[dc][:], dt16_sb[dc][:], xc_sb[dc][:], ALU.mult, "du")

                # ---- SSM scan: dc-pairs, y accumulated on PE into PSUM ----
                yg_sb = [big.tile([128, T], F16, tag=f"yg{dc}", name=f"yg{j}_{dc}") for dc in range(NDC)]
                for pair in range(NDC // 2):
                    dcs = (2 * pair, 2 * pair + 1)
                    py = {dc: pstile("Y0" if dc % 2 == 0 else "Y1", f"py{j}_{dc}") for dc in dcs}
                    for n in range(D_STATE):
                        brep = w16.tile([128, T], F16, tag="brep", name=f"brep{j}_{pair}_{n}", bufs=3)
                        nc.sync.dma_start(
                            brep[:], bcd[n : n + 1, :].partition_broadcast(128)
                        )
                        crep = w16.tile([128, T], F16, tag="crep", name=f"crep{j}_{pair}_{n}", bufs=3)
                        nc.sync.dma_start(
                            crep[:],
                            bcd[D_STATE + n : D_STATE + n + 1, :].partition_broadcast(128),
                        )
                        for dc in dcs:
                            dA = w16.tile([128, T], F16, tag=f"dA{dc % 2}", name=f"dA{j}_{n}_{dc}", bufs=3)
                            nc.scalar.activation(
                                dA[:], dt16_sb[dc][:], AF.Exp, scale=A_sb[dc][:, n : n + 1]
                            )
                            dBu = w16.tile([128, T], F16, tag="dBu", name=f"dBu{j}_{n}_{dc}", bufs=3)
                            tt(dBu[:], du16_sb[dc][:], brep[:], ALU.mult, "dBu")
                            h = w16.tile([128, T], F16, tag="h", name=f"h{j}_{n}_{dc}", bufs=3)
                            nc.vector.tensor_tensor_scan(
                                h[:], dA[:], dBu[:], 0.0, ALU.mult, ALU.add
                            )
                            hC = w16.tile([128, T], F16, tag="hC", name=f"hC{j}_{n}_{dc}", bufs=3)
                            tt(hC[:], h[:], crep[:], ALU.mult, "hC")
                            for nt in range(NT):
                                sl = slice(nt * 512, (nt + 1) * 512)
                                nc.tensor.matmul(
                                    py[dc][:, sl],
                                    identh_sb[:],
                                    hC[:, sl],
                                    start=(n == 0),
                                    stop=False,
                                )
                    for dc in dcs:
                        # D-skip rides the same accumulation: py += diag(D)*xc
                        for nt in range(NT):
                            sl = slice(nt * 512, (nt + 1) * 512)
                            nc.tensor.matmul(
                                py[dc][:, sl], dd_sb[dc][:], xc_sb[dc][:, sl],
                                start=False, stop=True,
                            )
                        # gate
                        nc.vector.tensor_tensor(
                            yg_sb[dc][:], py[dc][:], sz_sb[dc][:], ALU.mult
                        )

                # ---- out_proj + residual (residual rides PE as I @ u16) ----
                yr_sb = [sc32.tile([128, T], F32, tag=f"yr{mc}", name=f"yr{j}_{mc}") for mc in range(NMC)]
                sq_sb = [sc32.tile([128, T], F32, tag=f"sq{mc}", name=f"sq{j}_{mc}") for mc in range(NMC)]
                po_tiles = []
                for mc in range(NMC):
                    po = pstile("A0" if mc == 0 else "C0", f"po{j}_{mc}")
                    po_tiles.append(po)
                    for nt in range(NT):
                        sl = slice(nt * 512, (nt + 1) * 512)
                        for kc in range(NDC):
                            nc.tensor.matmul(
                                po[:, sl],
                                owT_sb[kc][:, mc * 128 : (mc + 1) * 128],
                                yg_sb[kc][:, sl],
                                start=(kc == 0),
                                stop=False,
                            )
                        nc.tensor.matmul(
                            po[:, sl], identh_sb[:], u16[mc][:, sl],
                            start=False, stop=True,
                        )
                    nc.scalar.copy(yr_sb[mc][:], po[:])
                    nc.scalar.activation(sq_sb[mc][:], yr_sb[mc][:], AF.Square)

                # ---- layernorm over the model dim (partition axis) ----
                s1 = pstile("Y0", f"s1{j}")
                s2 = pstile("Y1", f"s2{j}")
                mu_sb = cpool.tile([1, T], F32, name=f"mu{j}", tag="mu", bufs=1)
                ms_sb = cpool.tile([1, T], F32, name=f"ms{j}", tag="lnsm", bufs=3)
                for nt in range(NT):
                    sl = slice(nt * 512, (nt + 1) * 512)
                    for mc in range(NMC):
                        nc.tensor.matmul(
                            s1[0:1, sl], ones_col[:, 0:1], yr_sb[mc][:, sl],
                            start=(mc == 0), stop=(mc == NMC - 1),
                        )
                    for mc in range(NMC):
                        nc.tensor.matmul(
                            s2[0:1, sl], ones_col[:, 0:1], sq_sb[mc][:, sl],
                            start=(mc == 0), stop=(mc == NMC - 1),
                        )
                nc.scalar.mul(mu_sb[:], s1[0:1, :], 1.0 / D_MODEL)
                nc.scalar.mul(ms_sb[:], s2[0:1, :], 1.0 / D_MODEL)
                mu2_sb = cpool.tile([1, T], F32, name=f"mu2{j}", tag="lnsm", bufs=3)
                nc.scalar.activation(mu2_sb[:], mu_sb[:], AF.Square)
                var_sb = cpool.tile([1, T], F32, name=f"var{j}", tag="lnsm", bufs=3)
                nc.vector.tensor_tensor(var_sb[:], ms_sb[:], mu2_sb[:], ALU.subtract)
                # 1/sqrt(var+eps) = exp(-0.5*ln(var+eps)) — stays in the
                # ln/exp ACT table set
                sd_sb = cpool.tile([1, T], F32, name=f"sd{j}", tag="lnsm", bufs=3)
                nc.scalar.activation(sd_sb[:], var_sb[:], AF.Ln, bias=eps_sb[0:1, 0:1])
                inv_sb = cpool.tile([1, T], F32, name=f"inv{j}", tag="lnsm", bufs=3)
                nc.scalar.activation(inv_sb[:], sd_sb[:], AF.Exp, scale=-0.5)
                nmi_sb = cpool.tile([1, T], F32, name=f"nmi{j}", tag="lnsm", bufs=3)
                nc.vector.scalar_tensor_tensor(
                    nmi_sb[:], mu_sb[:], -1.0, inv_sb[:], ALU.mult, ALU.mult
                )
                pinv = pstile("Y0", f"pinv{j}")
                pnmi = pstile("Y1", f"pnmi{j}")
                for nt in range(NT):
                    sl = slice(nt * 512, (nt + 1) * 512)
                    nc.tensor.matmul(
                        pinv[:, sl], ones_row[0:1, :], inv_sb[0:1, sl],
                        start=True, stop=True,
                    )
                    nc.tensor.matmul(
                        pnmi[:, sl], ones_row[0:1, :], nmi_sb[0:1, sl],
                        start=True, stop=True,
                    )
                unew = [upool.tile([128, T], F16, tag=f"u{mc}", name=f"u{j}_{mc}") for mc in range(NMC)]
                utmp = [sc32.tile([128, T], F32, tag=f"ut{mc}", name=f"ut{j}_{mc}") for mc in range(NMC)]
                for mc in range(NMC):
                    nc.vector.tensor_tensor(utmp[mc][:], yr_sb[mc][:], pinv[:], ALU.mult)
                    nc.vector.tensor_tensor(utmp[mc][:], utmp[mc][:], pnmi[:], ALU.add)
                    nc.scalar.activation(
                        unew[mc][:],
                        utmp[mc][:],
                        AF.Identity,
                        bias=lnb_sb[mc][:, 0:1],
                        scale=lng_sb[mc][:, 0:1],
                    )
                return unew

            def emit_stack(u16s, joff):
                for i in range(NL):
                    u16s = emit_layer(u16s, joff + i)
                return u16s

            for _rep in range(repeat):
                x_f = emit_stack(load_x(False), 0)
                x_b_rev = emit_stack(load_x(True), NL)
                # reverse the bwd stack output back in time
                x_b = []
                for c in range(NMC):
                    tb = cpool.tile([128, T], F16, name=f"x_b{c}", tag="xio16", bufs=2)
                    nc.vector.tensor_copy(tb[:], x_b_rev[c][:, ::-1])
                    x_b.append(tb)

                # ---- fuse: out = fuse_w @ [x_f; x_b] + fb ----
                fwT_sb = []
                for kc in range(2 * NMC):
                    w = wpool.tile([128, D_MODEL], F16, name=f"fwT{kc}", tag=f"owT{kc}")
                    nc.sync.dma_start(w[:], fwTD.ap()[kc * 128 : (kc + 1) * 128, :])
                    fwT_sb.append(w)
                fb_sb = []
                for mc in range(NMC):
                    w = cpool.tile([128, 1], F32, name=f"fb{mc}", tag="fb", bufs=2)
                    nc.sync.dma_start(w[:], fbD.ap()[mc * 128 : (mc + 1) * 128, :])
                    fb_sb.append(w)
                xcat = x_f + x_b
                out_sb = []
                for mc in range(NMC):
                    pf = pstile("A0" if mc == 0 else "C0", f"pf{mc}")
                    for nt in range(NT):
                        for kc in range(2 * NMC):
                            nc.tensor.matmul(
                                pf[:, nt * 512 : (nt + 1) * 512],
                                fwT_sb[kc][:, mc * 128 : (mc + 1) * 128],
                                xcat[kc][:, nt * 512 : (nt + 1) * 512],
                                start=(kc == 0),
                                stop=(kc == 2 * NMC - 1),
                            )
                    o = sc32.tile([128, T], F32, name=f"out_sb{mc}", tag=f"yr{mc}")
                    nc.scalar.activation(o[:], pf[:], AF.Identity, bias=fb_sb[mc][:, 0:1])
                    out_sb.append(o)

                # ---- transpose to [T, D] and store ----
                pt_slots = [pstile("Y0", "ptY0"), pstile("Y1", "ptY1")]
                for tt_i in range(T // 128):
                    ot = cpool.tile([128, D_MODEL], F32, name=f"outT{tt_i}", tag="outT", bufs=4)
                    for mc in range(NMC):
                        pt = pt_slots[(tt_i * NMC + mc) % 2][:, 0:128]
                        nc.tensor.transpose(
                            pt, out_sb[mc][:, tt_i * 128 : (tt_i + 1) * 128], ident_sb[:]
                        )
                        nc.scalar.copy(ot[:, mc * 128 : (mc + 1) * 128], pt)
                    nc.sync.dma_start(outD.ap()[tt_i * 128 : (tt_i + 1) * 128, :], ot[:])

    if split_waits:
        _split_multi_waits(nc)
    return nc


_NC_CACHE = None


def _get_program():
    global _NC_CACHE
    if _NC_CACHE is None:
        _NC_CACHE = _build_program()
    return _NC_CACHE


def _prep_weights(inputs):
    f = np.float32
    iw = np.asarray(inputs["in_proj_w"], f)  # (4, 1024, 256)
    cw = np.asarray(inputs["conv_w"], f)  # (4, 512, 1, 4)
    cb = np.asarray(inputs["conv_b"], f)  # (4, 512)
    xw = np.asarray(inputs["xproj_w"], f)  # (4, 48, 512)
    dtw = np.asarray(inputs["dtproj_w"], f)  # (4, 512, 16)
    dtb = np.asarray(inputs["dtproj_b"], f)  # (4, 512)
    alog = np.asarray(inputs["A_log"], f)  # (4, 512, 16)
    Dp = np.asarray(inputs["D"], f)  # (4, 512)
    ow = np.asarray(inputs["out_w"], f)  # (4, 256, 512)
    lng = np.asarray(inputs["ln_g"], f)  # (4, 256)
    lnb = np.asarray(inputs["ln_b"], f)  # (4, 256)
    fw = np.asarray(inputs["fuse_w"], f)  # (256, 512)
    fb = np.asarray(inputs["fuse_b"], f)  # (256,)
    c = np.ascontiguousarray
    h = np.float16
    # conv taps and D as diag matrices for the PE (stationary operands)
    dcw = np.zeros((NB, NDC, D_CONV, 128, 128), h)
    dd = np.zeros((NB, NDC, 128, 128), h)
    idx = np.arange(128)
    for j in range(NB):
        for dc in range(NDC):
            dsl = slice(dc * 128, (dc + 1) * 128)
            for k in range(D_CONV):
                dcw[j, dc, k, idx, idx] = cw[j, dsl, 0, k].astype(h)
            dd[j, dc, idx, idx] = Dp[j, dsl].astype(h)
    return {
        "iwT": c(iw.transpose(0, 2, 1)).astype(h),  # (4, 256, 1024)
        "cb": c(cb[:, :, None]),
        "dcw": dcw,
        "dd": dd,
        "xwT": c(xw.transpose(0, 2, 1)).astype(h),  # (4, 512, 48)
        "dtwT": c(dtw.transpose(0, 2, 1)).astype(h),  # (4, 16, 512)
        "dtb": c(dtb[:, :, None]),
        "A": c(-np.exp(alog)),  # (4, 512, 16)
        "owT": c(ow.transpose(0, 2, 1)).astype(h),  # (4, 512, 256)
        "lng": c(lng[:, :, None]),
        "lnb": c(lnb[:, :, None]),
        "fwT": c(fw.T).astype(h),  # (512, 256)
        "fb": c(fb[:, None]),
        "ident": np.eye(128, dtype=f),
        "identh": np.eye(128, dtype=h),
    }


LAST_RUN = None


def kernel(**inputs) -> np.ndarray:
    global LAST_RUN
    import os

    nc = _get_program()
    x = np.asarray(inputs["x"], np.float32)  # (8, 1024, 256)
    assert x.shape == (NCORES, T, D_MODEL)
    w = _prep_weights(inputs)
    in_maps = [
        {"x16": np.ascontiguousarray(x[i]).astype(np.float16), **w}
        for i in range(NCORES)
    ]
    trace = bool(int(os.environ.get("BIMAMBA_TRACE", "0")))
    res = run_bass_kernel_spmd(
        nc, in_maps, core_ids=list(range(NCORES)), trace=trace
    )
    LAST_RUN = res
    out = np.stack([res.results[i]["out"] for i in range(NCORES)], axis=0)
    return out.astype(np.float32)


if __name__ == "__main__":
    # quick CoreSim numeric check against the jax reference
    import importlib.util
    import jax

    spec = importlib.util.spec_from_file_location("reference", "/root/problem/reference.py")
    ref = importlib.util.module_from_spec(spec)
    spec.loader.exec_module(ref)
    with jax.default_device(jax.devices("cpu")[0]):
        inputs = {k: np.asarray(v) for k, v in ref.setup_inputs().items()}
        expected = np.asarray(jax.jit(ref.reference, backend="cpu")(**inputs))

    from concourse.bass_interp import CoreSim, Direction, InstructionExecutor

    _orig_act = InstructionExecutor.visit_InstActivation

    def _patched_act(self, instruction, *args, **kwargs):
        f = instruction.func
        if f not in (AF.Silu, AF.Softplus):
            return _orig_act(self, instruction, *args, **kwargs)
        instruction.func = AF.Identity
        try:
            r = _orig_act(self, instruction, *args, **kwargs)
        finally:
            instruction.func = f
        out_ap = instruction.outs[0]
        view = self.view_ap(out_ap, Direction.WRITE, instruction)
        x = np.asarray(view[...], dtype=np.float64)
        if f == AF.Silu:
            y = x / (1.0 + np.exp(-x))
        else:
            y = np.logaddexp(0.0, x)
        view[:] = y
        return r

    InstructionExecutor.visit_InstActivation = _patched_act

    nc = _build_program(split_waits=False)
    w = _prep_weights(inputs)
    sim = CoreSim(nc)
    core = 0
    sim.tensor("x16")[:] = np.ascontiguousarray(inputs["x"][core]).astype(np.float16)
    for k, v in w.items():
        sim.tensor(k)[:] = v
    sim.simulate()
    got = sim.tensor("out")
    exp = expected[core]
    denom = np.abs(exp).max()
    err = np.abs(got - exp).max() / denom
    print("core0 absmax rel err:", err)
